# revision 3
# baseline (speedup 1.0000x reference)
import numpy as np
import jax

for _k, _v in (("jax_compilation_cache_dir", "/tmp/jax_cache"),
               ("jax_persistent_cache_min_compile_time_secs", 0.0),
               ("jax_persistent_cache_min_entry_size_bytes", -1)):
    try:
        jax.config.update(_k, _v)
    except Exception:
        pass

import jax.numpy as jnp

# Hardcoded problem shape (nn_AtomAttentionEncoderDiffusion):
#   D=8, L=2048, C_A=128, C_S=128, C_PAIR=16, H=4, c=32
# Sharding: data-parallel over diffusion batch D (one d per NeuronCore).
# The pair bias from Z_II is sequence-parallel: each core receives only the
# Z window-block slice for its 8 query windows, computes LN+projection there,
# and the [64,32,128,4] bias is assembled with an all-gather.
QB, KB = 32, 128
EPS = 1e-5
L = 2048
NQ = L // QB          # 64 query windows, contiguous 32-blocks
PAD = (KB - QB) // 2  # 48
ND = 8
WPD = NQ // ND        # 8 windows per device
CP = 16               # C_PAIR


def _ln(x, w=None, b=None):
    m = x.mean(-1, keepdims=True)
    v = x.var(-1, keepdims=True)
    y = (x - m) * jax.lax.rsqrt(v + EPS)
    if w is not None:
        y = y * w + b
    return y


def _key_mask():
    n = np.arange(NQ)[:, None]
    j = np.arange(KB)[None, :]
    pos = QB * n - PAD + j
    return (pos < 0) | (pos > L - 1)


_PENALTY = jnp.asarray(-1e9 * _key_mask()[:, None, :, None].astype(np.float32))


def _win_slices(x):
    def slc(n):
        return jax.lax.dynamic_slice_in_dim(x, n * QB, KB, axis=0)
    return jax.vmap(slc)(jnp.arange(NQ))


def _fwd(A, S, Zw, Wq, Wk, Wv, Wg, Wb_pair, ln0_w, ln0_b,
         ada_gW, ada_gb, ada_bW, Wa, Wo, bo):
    H, c = Wq.shape[1], Wq.shape[2]
    a = _ln(A)
    s = _ln(S)
    a = jax.nn.sigmoid(s @ ada_gW + ada_gb) * a + s @ ada_bW
    Q = jnp.einsum('lc,chk->lhk', a, Wq)
    K = jnp.einsum('lc,chk->lhk', a, Wk)
    V = jnp.einsum('lc,chk->lhk', a, Wv)
    G = jax.nn.sigmoid(jnp.einsum('lc,chk->lhk', a, Wg))

    # Zw: [WPD*QB, WPD*QB + KB - QB, CP] — this device's 8 windows of Z_II
    # (columns pre-padded/clamped on host). Local window m occupies rows
    # [32m, 32m+32) and columns [32m, 32m+128).
    def bias_block(m):
        zb = jax.lax.dynamic_slice(Zw, (m * QB, m * QB, 0), (QB, KB, CP))
        return jnp.einsum('ijp,ph->ijh', _ln(zb, ln0_w, ln0_b), Wb_pair)
    Bl = jax.vmap(bias_block)(jnp.arange(WPD))        # [WPD, QB, KB, H]
    Bb = jax.lax.all_gather(Bl, 'd', axis=0)          # [ND, WPD, QB, KB, H]
    Bb = Bb.reshape(NQ, QB, KB, H)

    qs = Q.reshape(NQ, QB, H, c)
    Kp = jnp.pad(K, ((PAD, PAD), (0, 0), (0, 0)))
    Vp = jnp.pad(V, ((PAD, PAD), (0, 0), (0, 0)))
    ks = _win_slices(Kp)  # [NQ, KB, H, c]
    vs = _win_slices(Vp)

    logits = jnp.einsum('nihc,njhc->nijh', qs, ks) / np.sqrt(c)
    logits = logits + Bb + _PENALTY
    attn = jax.nn.softmax(logits, axis=2)
    out = jnp.einsum('nijh,njhc->nihc', attn, vs)
    out = (G * out.reshape(L, H, c)).reshape(L, H * c)
    out = out @ Wa
    return jax.nn.sigmoid(S @ Wo + bo) * out


def kernel(A_I, S_I, Z_II, Wq, Wk, Wv, Wg, Wb_pair, ln0_w, ln0_b,
           ada_gW, ada_gb, ada_bW, Wa, Wo, bo):
    devs = jax.devices()[:ND]
    # Host-side slicing of Z_II: per device, rows [256k, 256k+256) and
    # edge-clamped columns [256k-48, 256k+304). Only devices 0 and ND-1
    # actually touch the clamped region, so pad just those slices.
    Z = np.asarray(Z_II)
    R = WPD * QB           # 256 rows per device
    W = R + KB - QB        # 352 cols per device
    Zw = np.empty((ND, R, W, CP), dtype=Z.dtype)
    for k in range(ND):
        lo, hi = k * R - PAD, k * R - PAD + W
        s = Z[k * R:(k + 1) * R, max(lo, 0):min(hi, L)]
        if lo < 0:
            s = np.concatenate([np.repeat(s[:, :1], -lo, axis=1), s], axis=1)
        if hi > L:
            s = np.concatenate([s, np.repeat(s[:, -1:], hi - L, axis=1)], axis=1)
        Zw[k] = s

    fn = jax.pmap(_fwd, axis_name='d', devices=devs,
                  in_axes=(0, 0, 0) + (None,) * 13)
    out = fn(jnp.asarray(A_I), jnp.asarray(S_I), jnp.asarray(Zw),
             jnp.asarray(Wq), jnp.asarray(Wk), jnp.asarray(Wv),
             jnp.asarray(Wg), jnp.asarray(Wb_pair), jnp.asarray(ln0_w),
             jnp.asarray(ln0_b), jnp.asarray(ada_gW), jnp.asarray(ada_gb),
             jnp.asarray(ada_bW), jnp.asarray(Wa), jnp.asarray(Wo),
             jnp.asarray(bo))
    return np.asarray(out).astype(np.float32)
